# revision 32
# baseline (speedup 1.0000x reference)
"""Trainium2 Bass kernel for nn_L1OutUB_14422500180350 (L1OutUB loss).

Math
----
reference computes, with B=512, Y=128 (see kernel_baseline.py for the
derivation of the [B,B,B] logsumexp collapse):
    mu     = relu(x @ w1_mu + b1_mu) @ w2_mu + b2_mu                  [B, Y]
    logvar = tanh(relu(x @ w1_lv + b1_lv) @ w2_lv + b2_lv)            [B, Y]
    iv     = exp(-logvar)
    s_pos_i = sum_k[(mu_ik - y_ik)^2 iv_ik + lv_ik]
    s_neg_i = sum_k[((mu_ik - my_k)^2 + vary_k) iv_ik + lv_ik]
    loss = -0.5*(mean_i s_pos_i - mean_i s_neg_i) - C,
    C = log(B-1+e^-20) - log(B-1)

The sum_k lv_ik terms cancel exactly in s_pos - s_neg, so each core
returns only
    pos_i  = sum_k (mu_ik - y_ik)^2 iv_ik
    negc_i = sum_k (mu_ik - my_k)^2 iv_ik + sum_k vary_k iv_ik
and the host combines loss = -0.5*(mean pos - mean negc) - C.

Distribution: data-parallel over batch rows, 64 rows/core on 8 cores;
weights replicated (spec sharding_hint). Everything transposed on-chip
(partition = feature dim) so biases / y-moments are per-partition
scalars and the final k-sums are ones-vector matmuls.

Precision: the loss is a small difference of large sums (|pos-neg| ~
s/300), so per-feature systematic rounding (biases, my, vary, lv)
amplifies ~300x. Numpy study (see transcript): bf16 x/w1/w2/h1 with an
f32 tail -> 3.5e-3 final rel err; bf16 anywhere in the tail -> 1e-2..
5e-2. Hence: bf16 for matmul inputs (they carry the DMA bytes), f32
for y/biases/moments and the whole D/Q/P/iv tail, float32r for the
reduce matmuls.

f32 tensors (consts, yT, ysT) ride inside the single uint16 "early"
DMA via AP.bitcast: the host packs raw f32/bf16 bytes into uint16
columns (uint16 so CoreSim's NaN input check ignores them).

Raw Bass (not Tile), one semaphore per DMA (HWDGE queues complete out
of order), explicit tick bookkeeping per engine.  The y moments are
computed on the Activation engine via accum_out (one Square + one
Identity pass over yT), R = [ysT | my broadcast] is prebuilt so the
whole D = (mu_psum + b2) - R is a single 128-wide DVE op reading the
L2 psum directly (no mu materialization), and the lv sums cancel so
the tail is just D -> Q=D*D -> P=Q*iv -> gpsimd cross-lane reduce ->
DMA.  The partition sum of P [128,192] (bf16, stacking Pp|Pn|vary*iv)
runs as gpsimd.tensor_reduce(axis=C) straight into SBUF -- it lives in
the standard GPSIMD ucode library so walrus codegens it, unlike
partition_all_reduce / kv_writeback+trigger_dma / f32r matmuls, which
all fail real-HW codegen ("ISA wrong length" / "DynamicDMA is
disabled" / FP32r rounding rules) despite passing CoreSim.  It also
replaces the PE ones-matmul + PSUM->SBUF copy pair, cutting two
engine hops off the critical tail.

PSUM banks: 0/1 L1-lv ping-pong, 2/3 L1-mu ping-pong, 4 L2-lv, 5 L2-mu.

Schedule notes (timeline-sim driven):
- DMA stream order = consumption order with the shortest post-chains
  last: early(consts/xs/yT/ysT) -> w1l.a -> w1m.a -> w1l.b -> w1m.b1
  -> w1m.b2 -> w2l -> w2m.  The stream is DMA_ENGINES-bandwidth-bound
  (~6.2us for 2.2MB at 360B/ns); w2m lands last because L2mu's
  post-chain (D->Q->P) is the shortest remaining work.
- Critical waits are fused onto instructions (wait_op) so the
  consumer sits pre-decoded in the engine queue when the sem fires.
- L2mu's accumulation group opens as soon as the early mu relus are
  done; only its last matmul waits on mu3's relu.
"""

from contextlib import ExitStack

import ml_dtypes
import numpy as np

import concourse.bass as bass
from concourse import mybir
from concourse.bass_utils import run_bass_kernel_spmd

B, X_DIM, Y_DIM, H2 = 512, 768, 128, 512
N_CORES = 8
RB = B // N_CORES  # 64 batch rows per core
KT = X_DIM // 128  # 6 k-tiles over the input dim
MT = H2 // 128  # 4 chunks over the hidden dim
F32 = mybir.dt.float32
BF16 = mybir.dt.bfloat16
F8E5 = mybir.dt.float8e5
U16 = mybir.dt.uint16
AF = mybir.ActivationFunctionType
ALU = mybir.AluOpType
AX = mybir.AxisListType

# early-tensor column layout (uint16 units).  yT ships as bf16 hi + fp8e5
# residual (reconstructed on-chip to ~1e-4 rel, enough for the y moments;
# ysT stays f32 for the (mu-y)^2 term) -- saves 512B/row of DMA stream.
C0 = 0  # consts f32 x11 -> cols [0,22)
XS0 = 24  # xs bf16 [6 ktiles x 64] -> [24,408)
YT0 = 408  # yT bf16-hi [512] -> [408,920)
YLO0 = 920  # yT fp8e5 residual [512] -> [920,1176)
YS0 = 1176  # ysT f32 [64] -> [1176,1304)
NE = 1304
# consts f32 column indices: 0-3 b1_mu chunks, 4 b2_mu, 5-8 b1_lv, 9 b2_lv, 10 ones
CB1M, CB2M, CB1L, CB2L, CONE = 0, 4, 5, 9, 10

# ACT ticks: 1 y-sq, 2 y-id, 3 R-pos copy, 4 R-neg (my broadcast),
# 5..11 relus (pS order), 12 tanh, 13 mu3-relu, 14 exp, 15 variv
A_YMOM = 2
A_RN = 4
A_MU3R = 12
A_TANH = 13
A_EXP = 14
A_VIV = 15
# DVE ticks
V_Y32, V_MY, V_MYSQ, V_VARY, V_D, V_Q, V_P = range(1, 8)
# Pool ticks (pPrep): 1 kv_idx memset, 2 allred-tail memset, 3 kvwb prep,
# 4 allreduce done
PP_ARED = 4
# PE ticks
P_LV0, P_LV1, P_MU0, P_MU1, P_LV2, P_LV3, P_MU2, P_MU3, P_L2LV, P_L2MU = range(1, 11)
# (head, chunk, pS tick, bank, ACT tick) for the 8 relus in pS order
RELU_ORDER = [
    ("l", 0, P_LV0, 0, 5),
    ("l", 1, P_LV1, 1, 6),
    ("m", 0, P_MU0, 2, 7),
    ("m", 1, P_MU1, 3, 8),
    ("l", 2, P_LV2, 0, 9),
    ("l", 3, P_LV3, 1, 10),
    ("m", 2, P_MU2, 2, 11),
    ("m", 3, P_MU3, 3, A_MU3R),
]


def build_nc() -> bass.Bass:
    nc = bass.Bass("TRN2", target_bir_lowering=False, debug=False)

    early_d = nc.dram_tensor("early", [128, NE], U16, kind="ExternalInput").ap()
    w1m_d = nc.dram_tensor("w1m", [128, KT * H2], BF16, kind="ExternalInput").ap()
    w1l_d = nc.dram_tensor("w1l", [128, KT * H2], BF16, kind="ExternalInput").ap()
    w2_d = nc.dram_tensor("w2", [128, 2 * 4 * Y_DIM], BF16, kind="ExternalInput").ap()
    out_d = nc.dram_tensor("out", [1, 4 * RB], F32, kind="ExternalOutput").ap()

    with ExitStack() as ctx:
        e = ctx.enter_context
        # ---- SBUF ----
        early = e(nc.sbuf_tensor("early_sb", [128, NE], U16))
        w1 = {
            "m": e(nc.sbuf_tensor("w1m_sb", [128, KT * H2], BF16)),
            "l": e(nc.sbuf_tensor("w1l_sb", [128, KT * H2], BF16)),
        }
        w2 = e(nc.sbuf_tensor("w2_sb", [128, 2 * 4 * Y_DIM], BF16))
        h1 = {
            "m": e(nc.sbuf_tensor("h1m_sb", [128, MT * RB], BF16)),
            "l": e(nc.sbuf_tensor("h1l_sb", [128, MT * RB], BF16)),
        }
        y32 = e(nc.sbuf_tensor("y32_sb", [128, B], F32))
        ydump1 = e(nc.sbuf_tensor("ydump1", [128, B], F32))
        ydump2 = e(nc.sbuf_tensor("ydump2", [128, B], F32))
        my_raw = e(nc.sbuf_tensor("my_raw", [128, 1], F32))
        my2_raw = e(nc.sbuf_tensor("my2_raw", [128, 1], F32))
        my = e(nc.sbuf_tensor("my_sb", [128, 1], F32))
        mysq = e(nc.sbuf_tensor("mysq_sb", [128, 1], F32))
        vary = e(nc.sbuf_tensor("vary_sb", [128, 1], F32))
        R = e(nc.sbuf_tensor("R_sb", [128, 2 * RB], F32))
        lvT = e(nc.sbuf_tensor("lvT_sb", [128, RB], F32))
        iv = e(nc.sbuf_tensor("iv_sb", [128, RB], F32))
        D = e(nc.sbuf_tensor("D_sb", [128, 2 * RB], F32))
        Q = e(nc.sbuf_tensor("Q_sb", [128, 2 * RB], F32))
        P = e(nc.sbuf_tensor("P_sb", [128, 3 * RB], BF16))
        allred = e(nc.sbuf_tensor("allred_sb", [128, 4 * RB], F32))
        kv_idx = e(nc.sbuf_tensor("kv_idx", [128, 1], I32))
        banks = [e(nc.psum_tensor(f"bank{i}", [128, 512], F32)) for i in range(7)]

        # f32/bf16 views into the early tensor
        consts = early[:, C0 : C0 + 22].bitcast(F32)  # [128, 11]
        y_hi = early[:, YT0 : YT0 + B].bitcast(BF16)  # [128, 512]
        y_lo = early[:, YLO0 : YLO0 + B // 2].bitcast(F8E5)  # [128, 512]
        ysT_f = early[:, YS0 : YS0 + 2 * RB].bitcast(F32)  # [128, 64]
        xs_t = lambda t: early[:, XS0 + t * RB : XS0 + (t + 1) * RB].bitcast(BF16)
        ccol = lambda k: consts[:, k : k + 1]

        dE = e(nc.semaphore("dE"))
        dLa = e(nc.semaphore("dLa"))
        dMa = e(nc.semaphore("dMa"))
        dW2l = e(nc.semaphore("dW2l"))
        dW2m = e(nc.semaphore("dW2m"))
        dLb = e(nc.semaphore("dLb"))
        dMb1 = e(nc.semaphore("dMb1"))
        dMb2 = e(nc.semaphore("dMb2"))
        dOut = e(nc.semaphore("dOut"))
        pPrep = e(nc.semaphore("pPrep"))
        pS = e(nc.semaphore("pS"))
        aS = e(nc.semaphore("aS"))
        vS = e(nc.semaphore("vS"))

        HB = KT * H2 // 2  # 1536: half of a packed w1 (chunks 0,1)
        CW = KT * 128  # 768: packed cols per chunk

        with nc.Block() as block:

            @block.sync
            def _(sync):
                sync.dma_start(out=early[:, :], in_=early_d).then_inc(dE, 16)
                sync.dma_start(out=w1["l"][:, 0:HB], in_=w1l_d[:, 0:HB]).then_inc(dLa, 16)
                sync.dma_start(out=w1["m"][:, 0:HB], in_=w1m_d[:, 0:HB]).then_inc(dMa, 16)
                sync.dma_start(out=w1["l"][:, HB:], in_=w1l_d[:, HB:]).then_inc(dLb, 16)
                sync.dma_start(
                    out=w1["m"][:, HB : HB + CW], in_=w1m_d[:, HB : HB + CW]
                ).then_inc(dMb1, 16)
                sync.dma_start(
                    out=w1["m"][:, HB + CW :], in_=w1m_d[:, HB + CW :]
                ).then_inc(dMb2, 16)
                sync.dma_start(out=w2[:, 512:], in_=w2_d[:, 512:]).then_inc(dW2l, 16)
                sync.dma_start(out=w2[:, 0:512], in_=w2_d[:, 0:512]).then_inc(dW2m, 16)

            @block.gpsimd
            def _(gpsimd):
                # Prepared-descriptor output DMA: out_sb[0, 0:128] -> out_d.
                # kv_writeback shapes: in [dhi=1, dho=128, batch=1, ncn=1],
                # out [batch=1, dhi=1, dho=128, nctx=1], ctx idx 0.
                from concourse import library_config

                from concourse import bass_isa

                nc.gpsimd.memset(kv_idx[:, :], 0).then_inc(pPrep, 1)
                nc.gpsimd.memset(allred[:, 3 * RB : 4 * RB], 0.0).then_inc(pPrep, 1)
                nc.gpsimd.load_library(library_config.attn)
                gpsimd.wait_ge(pPrep, 2)  # same-engine RAW: kv_idx visible
                in4 = allred[0:1, :].rearrange("p (a b c) -> p a b c", b=1, c=1)
                out4 = out_d.rearrange("a (b c d) -> a b c d", b=1, d=1)
                nc.gpsimd.kv_writeback(
                    out4, in4, kv_idx[:, :], prepare_only=True, sem=dOut
                ).then_inc(pPrep, 1)
                gpsimd.wait_ge(pPrep, 3)
                gpsimd.wait_ge(aS, A_VIV)
                nc.gpsimd.partition_all_reduce(
                    allred[:, 0 : 3 * RB], P[:, :], 128, bass_isa.ReduceOp.add
                ).wait_op(vS, V_P, "sem-ge").then_inc(pPrep, 1)  # PP_ARED
                nc.gpsimd.trigger_dma(count=1).wait_op(
                    pPrep, PP_ARED, "sem-ge"
                )
                gpsimd.wait_ge(dOut, 16)

            @block.gpsimd
            def _(gpsimd):
                gpsimd.wait_ge(aS, A_VIV)
                nc.gpsimd.tensor_reduce(
                    out_sb[:, :], P[:, :], axis=AX.C, op=ALU.add
                ).wait_op(vS, V_P, "sem-ge").then_inc(pS, 1)  # P_RED

            @block.tensor
            def _(tensor):
                def l1_chunk(head, m, bank):
                    for t in range(KT):
                        mm = nc.tensor.matmul(
                            bank[:, 0:RB],
                            w1[head][:, m * CW + t * 128 : m * CW + (t + 1) * 128],
                            xs_t(t),
                            start=(t == 0),
                            stop=(t == KT - 1),
                        )
                    mm.then_inc(pS, 1)

                def l2(head, bank, wait=None):
                    off = 512 if head == "l" else 0
                    for t in range(MT):
                        mm = nc.tensor.matmul(
                            bank[:, 0:RB],
                            w2[:, off + t * 128 : off + (t + 1) * 128],
                            h1[head][:, t * RB : (t + 1) * RB],
                            start=(t == 0),
                            stop=(t == MT - 1),
                        )
                        if t == 0 and wait is not None:
                            mm.wait_op(wait[0], wait[1], "sem-ge")
                    mm.then_inc(pS, 1)

                tensor.wait_ge(dE, 16)
                tensor.wait_ge(dLa, 16)
                l1_chunk("l", 0, banks[0])  # pS1
                l1_chunk("l", 1, banks[1])  # pS2
                tensor.wait_ge(dMa, 16)
                l1_chunk("m", 0, banks[2])  # pS3
                l1_chunk("m", 1, banks[3])  # pS4
                tensor.wait_ge(dLb, 16)
                tensor.wait_ge(aS, 5)  # relu lv0 done -> bank0 free
                l1_chunk("l", 2, banks[0])  # pS5
                tensor.wait_ge(aS, 6)
                l1_chunk("l", 3, banks[1])  # pS6
                tensor.wait_ge(dMb1, 16)
                tensor.wait_ge(aS, 7)  # relu mu0 -> bank2 free
                l1_chunk("m", 2, banks[2])  # pS7
                tensor.wait_ge(aS, 8)  # relu mu1 -> bank3 free
                for t in range(KT):
                    mm = nc.tensor.matmul(
                        banks[3][:, 0:RB],
                        w1["m"][:, 3 * CW + t * 128 : 3 * CW + (t + 1) * 128],
                        xs_t(t),
                        start=(t == 0),
                        stop=(t == KT - 1),
                    )
                    if t == 0:
                        mm.wait_op(dMb2, 16, "sem-ge")
                mm.then_inc(pS, 1)  # pS8 = P_MU3
                tensor.wait_ge(aS, 10)  # all lv relus
                l2("l", banks[4], wait=(dW2l, 16))  # pS9 = P_L2LV
                # L2mu: k-chunks 0-2 need only the early mu relus; the last
                # matmul alone waits on mu3's relu (fused), closing the group.
                tensor.wait_ge(aS, 11)  # mu2 relu
                for t in range(MT):
                    mm = nc.tensor.matmul(
                        banks[5][:, 0:RB],
                        w2[:, t * 128 : (t + 1) * 128],
                        h1["m"][:, t * RB : (t + 1) * RB],
                        start=(t == 0),
                        stop=(t == MT - 1),
                    )
                    if t == 0:
                        mm.wait_op(dW2m, 16, "sem-ge")
                    if t == MT - 1:
                        mm.wait_op(aS, A_MU3R, "sem-ge")
                mm.then_inc(pS, 1)  # pS10 = P_L2MU

            @block.scalar
            def _(scalar):
                scalar.wait_ge(dE, 16)
                scalar.wait_ge(vS, V_Y32)
                nc.scalar.activation(
                    out=ydump1[:, :], in_=y32[:, :], func=AF.Square,
                    accum_out=my2_raw[:, :],
                ).then_inc(aS, 1)  # 1
                nc.scalar.activation(
                    out=ydump2[:, :], in_=y32[:, :], func=AF.Identity,
                    accum_out=my_raw[:, :],
                ).then_inc(aS, 1)  # 2
                nc.scalar.activation(
                    out=R[:, 0:RB], in_=ysT_f, func=AF.Identity
                ).then_inc(aS, 1)  # 3: R pos half = ysT
                scalar.wait_ge(vS, V_MY)
                nc.scalar.activation(
                    out=R[:, RB : 2 * RB], in_=ysT_f, func=AF.Identity,
                    scale=0.0, bias=my[:, :],
                ).then_inc(aS, 1)  # A_RN: R neg half = my broadcast
                for head, m, ps_tick, bank, a_tick in RELU_ORDER:
                    bcol = (CB1L if head == "l" else CB1M) + m
                    nc.scalar.activation(
                        out=h1[head][:, m * RB : (m + 1) * RB],
                        in_=banks[bank][:, 0:RB],
                        func=AF.Relu,
                        bias=ccol(bcol),
                        scale=1.0,
                    ).wait_op(pS, ps_tick, "sem-ge").then_inc(aS, 1)  # 5..11
                nc.scalar.activation(
                    out=lvT[:, :],
                    in_=banks[4][:, 0:RB],
                    func=AF.Tanh,
                    bias=ccol(CB2L),
                ).wait_op(pS, P_L2LV, "sem-ge").then_inc(aS, 1)  # A_TANH
                nc.scalar.activation(
                    out=iv[:, :], in_=lvT[:, :], func=AF.Exp, scale=-1.0
                ).wait_op(aS, A_TANH, "sem-ge").then_inc(aS, 1)  # A_EXP
                scalar.wait_ge(vS, V_VARY)
                nc.scalar.activation(
                    out=P[:, 2 * RB : 3 * RB],
                    in_=iv[:, :],
                    func=AF.Identity,
                    scale=vary[:, :],
                ).wait_op(aS, A_EXP, "sem-ge").then_inc(aS, 1)  # A_VIV

            @block.vector
            def _(vector):
                vector.wait_ge(dE, 16)
                nc.vector.tensor_add(y32[:, :], y_hi, y_lo).then_inc(vS, 1)  # V_Y32
                vector.wait_ge(aS, A_YMOM)
                nc.vector.tensor_scalar_mul(my[:, :], my_raw[:, :], 1.0 / B).then_inc(
                    vS, 1
                )  # V_MY
                vector.wait_ge(vS, V_MY)
                nc.vector.tensor_mul(mysq[:, :], my[:, :], my[:, :]).then_inc(vS, 1)
                vector.wait_ge(vS, V_MYSQ)
                nc.vector.scalar_tensor_tensor(
                    out=vary[:, :],
                    in0=my2_raw[:, :],
                    scalar=1.0 / B,
                    in1=mysq[:, :],
                    op0=ALU.mult,
                    op1=ALU.subtract,
                ).then_inc(vS, 1)  # V_VARY
                vector.wait_ge(aS, A_RN)  # R fully built
                bank5b = (
                    banks[5][:, 0:RB]
                    .rearrange("p (r c) -> p r c", r=1)
                    .broadcast_to([128, 2, RB])
                )
                r3 = R[:, :].rearrange("p (r c) -> p r c", r=2)
                d3 = D[:, :].rearrange("p (r c) -> p r c", r=2)
                nc.vector.scalar_tensor_tensor(
                    out=d3,
                    in0=bank5b,
                    scalar=ccol(CB2M),
                    in1=r3,
                    op0=ALU.add,
                    op1=ALU.subtract,
                ).wait_op(pS, P_L2MU, "sem-ge").then_inc(vS, 1)  # V_D
                nc.vector.tensor_mul(Q[:, :], D[:, :], D[:, :]).wait_op(
                    vS, V_D, "sem-ge"
                ).then_inc(vS, 1)  # V_Q
                vector.wait_ge(aS, A_EXP)
                q3 = Q[:, :].rearrange("p (r c) -> p r c", r=2)
                p3 = P[:, 0 : 2 * RB].rearrange("p (r c) -> p r c", r=2)
                ivb = (
                    iv[:, :]
                    .rearrange("p (r c) -> p r c", r=1)
                    .broadcast_to([128, 2, RB])
                )
                nc.vector.tensor_mul(p3, q3, ivb).wait_op(
                    vS, V_Q, "sem-ge"
                ).then_inc(vS, 1)  # V_P

    return nc


def make_in_maps(inputs: dict) -> list[dict]:
    f32 = lambda a: np.ascontiguousarray(np.asarray(a, dtype=np.float32))
    bf = lambda a: np.ascontiguousarray(np.asarray(a, dtype=np.float32)).astype(
        ml_dtypes.bfloat16
    )
    x = f32(inputs["x_samples"])  # [512, 768]
    y = f32(inputs["y_samples"])  # [512, 128]
    yT = f32(y.T)  # [128, 512]

    consts = np.zeros((128, 11), np.float32)
    consts[:, CB1M : CB1M + 4] = f32(inputs["b1_mu"]).reshape(4, 128).T
    consts[:, CB2M] = f32(inputs["b2_mu"])
    consts[:, CB1L : CB1L + 4] = f32(inputs["b1_lv"]).reshape(4, 128).T
    consts[:, CB2L] = f32(inputs["b2_lv"])
    consts[:, CONE] = 1.0

    def pack_w1(w):  # [768, 512] -> [128, 3072] chunk-major bf16
        a = bf(w).reshape(KT, 128, MT, 128)  # [t, p, m, c]
        return np.ascontiguousarray(
            a.transpose(1, 2, 0, 3).reshape(128, MT * KT * 128)
        )

    def pack_w2(w):  # [512, 128] -> [128, 512]
        a = bf(w).reshape(MT, 128, Y_DIM)
        return np.ascontiguousarray(a.transpose(1, 0, 2).reshape(128, MT * Y_DIM))

    w1m_p = pack_w1(inputs["w1_mu"])
    w1l_p = pack_w1(inputs["w1_lv"])
    w2_p = np.concatenate([pack_w2(inputs["w2_mu"]), pack_w2(inputs["w2_lv"])], axis=1)

    in_maps = []
    for c in range(N_CORES):
        sl = slice(c * RB, (c + 1) * RB)
        xs = x[sl]  # [64, 768]
        xp = np.ascontiguousarray(
            xs.T.reshape(KT, 128, RB).transpose(1, 0, 2).reshape(128, KT * RB)
        ).astype(ml_dtypes.bfloat16)
        ysT = f32(y[sl].T)  # [128, 64]

        y_hi = yT.astype(ml_dtypes.bfloat16)
        y_lo = (yT - y_hi.astype(np.float32)).astype(ml_dtypes.float8_e5m2)
        early = np.zeros((128, NE), np.uint16)
        early[:, C0 : C0 + 22] = consts.view(np.uint16)
        early[:, XS0 : XS0 + KT * RB] = xp.view(np.uint16)
        early[:, YT0 : YT0 + B] = y_hi.view(np.uint16)
        early[:, YLO0 : YLO0 + B // 2] = np.ascontiguousarray(y_lo).view(np.uint16)
        early[:, YS0 : YS0 + 2 * RB] = ysT.view(np.uint16)

        in_maps.append(
            {
                "early": early,
                "w1m": w1m_p,
                "w1l": w1l_p,
                "w2": w2_p,
            }
        )
    return in_maps


def combine(results: list[dict]) -> np.float32:
    outs = [np.asarray(results[c]["out"], np.float64)[0] for c in range(N_CORES)]
    pos = np.concatenate([o[:RB] for o in outs])
    negc = np.concatenate([o[RB : 2 * RB] + o[2 * RB : 3 * RB] for o in outs])
    C = np.log(B - 1.0 + np.exp(-20.0)) - np.log(B - 1.0)
    loss = -0.5 * (pos.mean() - negc.mean()) - C
    return np.float32(loss)


_NC_CACHE = None


def run(inputs: dict, **spmd_kwargs):
    """Build (cached), run on 8 cores, return (loss, BassKernelResults)."""
    global _NC_CACHE
    if _NC_CACHE is None:
        _NC_CACHE = build_nc()
    bkr = run_bass_kernel_spmd(
        _NC_CACHE, make_in_maps(inputs), list(range(N_CORES)), **spmd_kwargs
    )
    return combine(bkr.results), bkr


def kernel(**inputs) -> np.float32:
    loss, _ = run(inputs)
    return loss


# revision 34
# speedup vs baseline: 1.0016x; 1.0016x over previous
"""Trainium2 Bass kernel for nn_L1OutUB_14422500180350 (L1OutUB loss).

Math
----
reference computes, with B=512, Y=128 (see kernel_baseline.py for the
derivation of the [B,B,B] logsumexp collapse):
    mu     = relu(x @ w1_mu + b1_mu) @ w2_mu + b2_mu                  [B, Y]
    logvar = tanh(relu(x @ w1_lv + b1_lv) @ w2_lv + b2_lv)            [B, Y]
    iv     = exp(-logvar)
    s_pos_i = sum_k[(mu_ik - y_ik)^2 iv_ik + lv_ik]
    s_neg_i = sum_k[((mu_ik - my_k)^2 + vary_k) iv_ik + lv_ik]
    loss = -0.5*(mean_i s_pos_i - mean_i s_neg_i) - C,
    C = log(B-1+e^-20) - log(B-1)

The sum_k lv_ik terms cancel exactly in s_pos - s_neg, so each core
returns only
    pos_i  = sum_k (mu_ik - y_ik)^2 iv_ik
    negc_i = sum_k (mu_ik - my_k)^2 iv_ik + sum_k vary_k iv_ik
and the host combines loss = -0.5*(mean pos - mean negc) - C.

Distribution: data-parallel over batch rows, 64 rows/core on 8 cores;
weights replicated (spec sharding_hint). Everything transposed on-chip
(partition = feature dim) so biases / y-moments are per-partition
scalars and the final k-sums are ones-vector matmuls.

Precision: the loss is a small difference of large sums (|pos-neg| ~
s/300), so per-feature systematic rounding (biases, my, vary, lv)
amplifies ~300x. Numpy study (see transcript): bf16 x/w1/w2/h1 with an
f32 tail -> 3.5e-3 final rel err; bf16 anywhere in the tail -> 1e-2..
5e-2. Hence: bf16 for matmul inputs (they carry the DMA bytes), f32
for y/biases/moments and the whole D/Q/P/iv tail, float32r for the
reduce matmuls.

f32 tensors (consts, yT, ysT) ride inside the single uint16 "early"
DMA via AP.bitcast: the host packs raw f32/bf16 bytes into uint16
columns (uint16 so CoreSim's NaN input check ignores them).

Raw Bass (not Tile), one semaphore per DMA (HWDGE queues complete out
of order), explicit tick bookkeeping per engine.  The y moments are
computed on the Activation engine via accum_out (one Square + one
Identity pass over yT), R = [ysT | my broadcast] is prebuilt so the
whole D = (mu_psum + b2) - R is a single 128-wide DVE op reading the
L2 psum directly (no mu materialization), and the lv sums cancel so
the tail is just D -> Q=D*D -> P=Q*iv -> gpsimd cross-lane reduce ->
DMA.  The partition sum of P [128,192] (bf16, stacking Pp|Pn|vary*iv)
runs as gpsimd.tensor_reduce(axis=C) straight into SBUF -- it lives in
the standard GPSIMD ucode library so walrus codegens it, unlike
partition_all_reduce / kv_writeback+trigger_dma / f32r matmuls, which
all fail real-HW codegen ("ISA wrong length" / "DynamicDMA is
disabled" / FP32r rounding rules) despite passing CoreSim.  It also
replaces the PE ones-matmul + PSUM->SBUF copy pair, cutting two
engine hops off the critical tail.

PSUM banks: 0/1 L1-lv ping-pong, 2/3 L1-mu ping-pong, 4 L2-lv, 5 L2-mu.

Schedule notes (timeline-sim driven):
- DMA stream order = consumption order with the shortest post-chains
  last: early(consts/xs/yT/ysT) -> w1l.a -> w1m.a -> w1l.b -> w1m.b1
  -> w1m.b2 -> w2l -> w2m.  The stream is DMA_ENGINES-bandwidth-bound
  (~6.2us for 2.2MB at 360B/ns); w2m lands last because L2mu's
  post-chain (D->Q->P) is the shortest remaining work.
- Critical waits are fused onto instructions (wait_op) so the
  consumer sits pre-decoded in the engine queue when the sem fires.
- L2mu's accumulation group opens as soon as the early mu relus are
  done; only its last matmul waits on mu3's relu.
"""

from contextlib import ExitStack

import ml_dtypes
import numpy as np

import concourse.bass as bass
from concourse import mybir
from concourse.bass_utils import run_bass_kernel_spmd

B, X_DIM, Y_DIM, H2 = 512, 768, 128, 512
N_CORES = 8
RB = B // N_CORES  # 64 batch rows per core
KT = X_DIM // 128  # 6 k-tiles over the input dim
MT = H2 // 128  # 4 chunks over the hidden dim
F32 = mybir.dt.float32
BF16 = mybir.dt.bfloat16
F8E5 = mybir.dt.float8e5
U16 = mybir.dt.uint16
AF = mybir.ActivationFunctionType
ALU = mybir.AluOpType
AX = mybir.AxisListType

# early-tensor column layout (uint16 units).  yT ships as bf16 hi + fp8e5
# residual (reconstructed on-chip to ~1e-4 rel, enough for the y moments;
# ysT stays f32 for the (mu-y)^2 term) -- saves 512B/row of DMA stream.
C0 = 0  # consts f32 x11 -> cols [0,22)
XS0 = 24  # xs bf16 [6 ktiles x 64] -> [24,408)
YT0 = 408  # yT bf16-hi [512] -> [408,920)
YLO0 = 920  # yT fp8e5 residual [512] -> [920,1176)
YS0 = 1176  # ysT bf16-hi [64] -> [1176,1240)
YSLO0 = 1240  # ysT fp8e5 residual [64] -> [1240,1272)
NE = 1272
# consts f32 column indices: 0-3 b1_mu chunks, 4 b2_mu, 5-8 b1_lv, 9 b2_lv, 10 ones
CB1M, CB2M, CB1L, CB2L, CONE = 0, 4, 5, 9, 10

# ACT ticks: 1 y-sq, 2 y-id, 3 R-pos copy, 4 R-neg (my broadcast),
# 5..11 relus (pS order), 12 tanh, 13 mu3-relu, 14 exp, 15 variv
A_YMOM = 2
A_RN = 4
A_MU3R = 12
A_TANH = 13
A_EXP = 14
A_VIV = 15
# DVE ticks
V_Y32, V_YS32, V_MY, V_MYSQ, V_VARY, V_D, V_Q, V_P = range(1, 9)
# Pool ticks (pPrep): 1 kv_idx memset, 2 allred-tail memset, 3 kvwb prep,
# 4 allreduce done
PP_ARED = 4
# PE ticks
P_LV0, P_LV1, P_MU0, P_MU1, P_LV2, P_LV3, P_MU2, P_MU3, P_L2LV, P_L2MU = range(1, 11)
# (head, chunk, pS tick, bank, ACT tick) for the 8 relus in pS order
RELU_ORDER = [
    ("l", 0, P_LV0, 0, 5),
    ("l", 1, P_LV1, 1, 6),
    ("m", 0, P_MU0, 2, 7),
    ("m", 1, P_MU1, 3, 8),
    ("l", 2, P_LV2, 0, 9),
    ("l", 3, P_LV3, 1, 10),
    ("m", 2, P_MU2, 2, 11),
    ("m", 3, P_MU3, 3, A_MU3R),
]


def build_nc() -> bass.Bass:
    nc = bass.Bass("TRN2", target_bir_lowering=False, debug=False)

    early_d = nc.dram_tensor("early", [128, NE], U16, kind="ExternalInput").ap()
    w1m_d = nc.dram_tensor("w1m", [128, KT * H2], BF16, kind="ExternalInput").ap()
    w1l_d = nc.dram_tensor("w1l", [128, KT * H2], BF16, kind="ExternalInput").ap()
    w2_d = nc.dram_tensor("w2", [128, 2 * 4 * Y_DIM], BF16, kind="ExternalInput").ap()
    out_d = nc.dram_tensor("out", [1, 4 * RB], F32, kind="ExternalOutput").ap()

    with ExitStack() as ctx:
        e = ctx.enter_context
        # ---- SBUF ----
        early = e(nc.sbuf_tensor("early_sb", [128, NE], U16))
        w1 = {
            "m": e(nc.sbuf_tensor("w1m_sb", [128, KT * H2], BF16)),
            "l": e(nc.sbuf_tensor("w1l_sb", [128, KT * H2], BF16)),
        }
        w2 = e(nc.sbuf_tensor("w2_sb", [128, 2 * 4 * Y_DIM], BF16))
        h1 = {
            "m": e(nc.sbuf_tensor("h1m_sb", [128, MT * RB], BF16)),
            "l": e(nc.sbuf_tensor("h1l_sb", [128, MT * RB], BF16)),
        }
        y32 = e(nc.sbuf_tensor("y32_sb", [128, B], F32))
        ys32 = e(nc.sbuf_tensor("ys32_sb", [128, RB], F32))
        ydump1 = e(nc.sbuf_tensor("ydump1", [128, B], F32))
        ydump2 = e(nc.sbuf_tensor("ydump2", [128, B], F32))
        my_raw = e(nc.sbuf_tensor("my_raw", [128, 1], F32))
        my2_raw = e(nc.sbuf_tensor("my2_raw", [128, 1], F32))
        my = e(nc.sbuf_tensor("my_sb", [128, 1], F32))
        mysq = e(nc.sbuf_tensor("mysq_sb", [128, 1], F32))
        vary = e(nc.sbuf_tensor("vary_sb", [128, 1], F32))
        R = e(nc.sbuf_tensor("R_sb", [128, 2 * RB], F32))
        lvT = e(nc.sbuf_tensor("lvT_sb", [128, RB], F32))
        iv = e(nc.sbuf_tensor("iv_sb", [128, RB], F32))
        D = e(nc.sbuf_tensor("D_sb", [128, 2 * RB], F32))
        Q = e(nc.sbuf_tensor("Q_sb", [128, 2 * RB], F32))
        P = e(nc.sbuf_tensor("P_sb", [128, 3 * RB], BF16))
        allred = e(nc.sbuf_tensor("allred_sb", [128, 4 * RB], F32))
        kv_idx = e(nc.sbuf_tensor("kv_idx", [128, 1], I32))
        banks = [e(nc.psum_tensor(f"bank{i}", [128, 512], F32)) for i in range(7)]

        # f32/bf16 views into the early tensor
        consts = early[:, C0 : C0 + 22].bitcast(F32)  # [128, 11]
        y_hi = early[:, YT0 : YT0 + B].bitcast(BF16)  # [128, 512]
        y_lo = early[:, YLO0 : YLO0 + B // 2].bitcast(F8E5)  # [128, 512]
        ys_hi = early[:, YS0 : YS0 + RB].bitcast(BF16)  # [128, 64]
        ys_lo = early[:, YSLO0 : YSLO0 + RB // 2].bitcast(F8E5)  # [128, 64]
        xs_t = lambda t: early[:, XS0 + t * RB : XS0 + (t + 1) * RB].bitcast(BF16)
        ccol = lambda k: consts[:, k : k + 1]

        dE = e(nc.semaphore("dE"))
        dLa = e(nc.semaphore("dLa"))
        dMa = e(nc.semaphore("dMa"))
        dW2l = e(nc.semaphore("dW2l"))
        dW2m = e(nc.semaphore("dW2m"))
        dLb = e(nc.semaphore("dLb"))
        dMb1 = e(nc.semaphore("dMb1"))
        dMb2 = e(nc.semaphore("dMb2"))
        dOut = e(nc.semaphore("dOut"))
        pPrep = e(nc.semaphore("pPrep"))
        pS = e(nc.semaphore("pS"))
        aS = e(nc.semaphore("aS"))
        vS = e(nc.semaphore("vS"))

        HB = KT * H2 // 2  # 1536: half of a packed w1 (chunks 0,1)
        CW = KT * 128  # 768: packed cols per chunk

        with nc.Block() as block:

            @block.sync
            def _(sync):
                sync.dma_start(out=early[:, :], in_=early_d).then_inc(dE, 16)
                sync.dma_start(out=w1["l"][:, 0:HB], in_=w1l_d[:, 0:HB]).then_inc(dLa, 16)
                sync.dma_start(out=w1["m"][:, 0:HB], in_=w1m_d[:, 0:HB]).then_inc(dMa, 16)
                sync.dma_start(out=w1["l"][:, HB:], in_=w1l_d[:, HB:]).then_inc(dLb, 16)
                sync.dma_start(
                    out=w1["m"][:, HB : HB + CW], in_=w1m_d[:, HB : HB + CW]
                ).then_inc(dMb1, 16)
                sync.dma_start(
                    out=w1["m"][:, HB + CW :], in_=w1m_d[:, HB + CW :]
                ).then_inc(dMb2, 16)
                sync.dma_start(out=w2[:, 512:], in_=w2_d[:, 512:]).then_inc(dW2l, 16)
                sync.dma_start(out=w2[:, 0:512], in_=w2_d[:, 0:512]).then_inc(dW2m, 16)

            @block.gpsimd
            def _(gpsimd):
                # Prepared-descriptor output DMA: out_sb[0, 0:128] -> out_d.
                # kv_writeback shapes: in [dhi=1, dho=128, batch=1, ncn=1],
                # out [batch=1, dhi=1, dho=128, nctx=1], ctx idx 0.
                from concourse import library_config

                from concourse import bass_isa

                nc.gpsimd.memset(kv_idx[:, :], 0).then_inc(pPrep, 1)
                nc.gpsimd.memset(allred[:, 3 * RB : 4 * RB], 0.0).then_inc(pPrep, 1)
                nc.gpsimd.load_library(library_config.attn)
                gpsimd.wait_ge(pPrep, 2)  # same-engine RAW: kv_idx visible
                in4 = allred[0:1, :].rearrange("p (a b c) -> p a b c", b=1, c=1)
                out4 = out_d.rearrange("a (b c d) -> a b c d", b=1, d=1)
                nc.gpsimd.kv_writeback(
                    out4, in4, kv_idx[:, :], prepare_only=True, sem=dOut
                ).then_inc(pPrep, 1)
                gpsimd.wait_ge(pPrep, 3)
                gpsimd.wait_ge(aS, A_VIV)
                nc.gpsimd.partition_all_reduce(
                    allred[:, 0 : 3 * RB], P[:, :], 128, bass_isa.ReduceOp.add
                ).wait_op(vS, V_P, "sem-ge").then_inc(pPrep, 1)  # PP_ARED
                nc.gpsimd.trigger_dma(count=1).wait_op(
                    pPrep, PP_ARED, "sem-ge"
                )
                gpsimd.wait_ge(dOut, 16)

            @block.gpsimd
            def _(gpsimd):
                gpsimd.wait_ge(aS, A_VIV)
                nc.gpsimd.tensor_reduce(
                    out_sb[:, :], P[:, :], axis=AX.C, op=ALU.add
                ).wait_op(vS, V_P, "sem-ge").then_inc(pS, 1)  # P_RED

            @block.tensor
            def _(tensor):
                def l1_chunk(head, m, bank):
                    for t in range(KT):
                        mm = nc.tensor.matmul(
                            bank[:, 0:RB],
                            w1[head][:, m * CW + t * 128 : m * CW + (t + 1) * 128],
                            xs_t(t),
                            start=(t == 0),
                            stop=(t == KT - 1),
                        )
                    mm.then_inc(pS, 1)

                def l2(head, bank, wait=None):
                    off = 512 if head == "l" else 0
                    for t in range(MT):
                        mm = nc.tensor.matmul(
                            bank[:, 0:RB],
                            w2[:, off + t * 128 : off + (t + 1) * 128],
                            h1[head][:, t * RB : (t + 1) * RB],
                            start=(t == 0),
                            stop=(t == MT - 1),
                        )
                        if t == 0 and wait is not None:
                            mm.wait_op(wait[0], wait[1], "sem-ge")
                    mm.then_inc(pS, 1)

                tensor.wait_ge(dE, 16)
                tensor.wait_ge(dLa, 16)
                l1_chunk("l", 0, banks[0])  # pS1
                l1_chunk("l", 1, banks[1])  # pS2
                tensor.wait_ge(dMa, 16)
                l1_chunk("m", 0, banks[2])  # pS3
                l1_chunk("m", 1, banks[3])  # pS4
                tensor.wait_ge(dLb, 16)
                tensor.wait_ge(aS, 5)  # relu lv0 done -> bank0 free
                l1_chunk("l", 2, banks[0])  # pS5
                tensor.wait_ge(aS, 6)
                l1_chunk("l", 3, banks[1])  # pS6
                tensor.wait_ge(dMb1, 16)
                tensor.wait_ge(aS, 7)  # relu mu0 -> bank2 free
                l1_chunk("m", 2, banks[2])  # pS7
                tensor.wait_ge(aS, 8)  # relu mu1 -> bank3 free
                for t in range(KT):
                    mm = nc.tensor.matmul(
                        banks[3][:, 0:RB],
                        w1["m"][:, 3 * CW + t * 128 : 3 * CW + (t + 1) * 128],
                        xs_t(t),
                        start=(t == 0),
                        stop=(t == KT - 1),
                    )
                    if t == 0:
                        mm.wait_op(dMb2, 16, "sem-ge")
                mm.then_inc(pS, 1)  # pS8 = P_MU3
                tensor.wait_ge(aS, 10)  # all lv relus
                l2("l", banks[4], wait=(dW2l, 16))  # pS9 = P_L2LV
                # L2mu: k-chunks 0-2 need only the early mu relus; the last
                # matmul alone waits on mu3's relu (fused), closing the group.
                tensor.wait_ge(aS, 11)  # mu2 relu
                for t in range(MT):
                    mm = nc.tensor.matmul(
                        banks[5][:, 0:RB],
                        w2[:, t * 128 : (t + 1) * 128],
                        h1["m"][:, t * RB : (t + 1) * RB],
                        start=(t == 0),
                        stop=(t == MT - 1),
                    )
                    if t == 0:
                        mm.wait_op(dW2m, 16, "sem-ge")
                    if t == MT - 1:
                        mm.wait_op(aS, A_MU3R, "sem-ge")
                mm.then_inc(pS, 1)  # pS10 = P_L2MU

            @block.scalar
            def _(scalar):
                scalar.wait_ge(dE, 16)
                scalar.wait_ge(vS, V_Y32)
                nc.scalar.activation(
                    out=ydump1[:, :], in_=y32[:, :], func=AF.Square,
                    accum_out=my2_raw[:, :],
                ).then_inc(aS, 1)  # 1
                nc.scalar.activation(
                    out=ydump2[:, :], in_=y32[:, :], func=AF.Identity,
                    accum_out=my_raw[:, :],
                ).then_inc(aS, 1)  # 2
                scalar.wait_ge(vS, V_YS32)
                nc.scalar.activation(
                    out=R[:, 0:RB], in_=ys32[:, :], func=AF.Identity
                ).then_inc(aS, 1)  # 3: R pos half = ysT
                scalar.wait_ge(vS, V_MY)
                nc.scalar.activation(
                    out=R[:, RB : 2 * RB], in_=ys32[:, :], func=AF.Identity,
                    scale=0.0, bias=my[:, :],
                ).then_inc(aS, 1)  # A_RN: R neg half = my broadcast
                for head, m, ps_tick, bank, a_tick in RELU_ORDER:
                    bcol = (CB1L if head == "l" else CB1M) + m
                    nc.scalar.activation(
                        out=h1[head][:, m * RB : (m + 1) * RB],
                        in_=banks[bank][:, 0:RB],
                        func=AF.Relu,
                        bias=ccol(bcol),
                        scale=1.0,
                    ).wait_op(pS, ps_tick, "sem-ge").then_inc(aS, 1)  # 5..11
                nc.scalar.activation(
                    out=lvT[:, :],
                    in_=banks[4][:, 0:RB],
                    func=AF.Tanh,
                    bias=ccol(CB2L),
                ).wait_op(pS, P_L2LV, "sem-ge").then_inc(aS, 1)  # A_TANH
                nc.scalar.activation(
                    out=iv[:, :], in_=lvT[:, :], func=AF.Exp, scale=-1.0
                ).wait_op(aS, A_TANH, "sem-ge").then_inc(aS, 1)  # A_EXP
                scalar.wait_ge(vS, V_VARY)
                nc.scalar.activation(
                    out=P[:, 2 * RB : 3 * RB],
                    in_=iv[:, :],
                    func=AF.Identity,
                    scale=vary[:, :],
                ).wait_op(aS, A_EXP, "sem-ge").then_inc(aS, 1)  # A_VIV

            @block.vector
            def _(vector):
                vector.wait_ge(dE, 16)
                nc.vector.tensor_add(y32[:, :], y_hi, y_lo).then_inc(vS, 1)  # V_Y32
                nc.vector.tensor_add(ys32[:, :], ys_hi, ys_lo).then_inc(
                    vS, 1
                )  # V_YS32
                vector.wait_ge(aS, A_YMOM)
                nc.vector.tensor_scalar_mul(my[:, :], my_raw[:, :], 1.0 / B).then_inc(
                    vS, 1
                )  # V_MY
                vector.wait_ge(vS, V_MY)
                nc.vector.tensor_mul(mysq[:, :], my[:, :], my[:, :]).then_inc(vS, 1)
                vector.wait_ge(vS, V_MYSQ)
                nc.vector.scalar_tensor_tensor(
                    out=vary[:, :],
                    in0=my2_raw[:, :],
                    scalar=1.0 / B,
                    in1=mysq[:, :],
                    op0=ALU.mult,
                    op1=ALU.subtract,
                ).then_inc(vS, 1)  # V_VARY
                vector.wait_ge(aS, A_RN)  # R fully built
                bank5b = (
                    banks[5][:, 0:RB]
                    .rearrange("p (r c) -> p r c", r=1)
                    .broadcast_to([128, 2, RB])
                )
                r3 = R[:, :].rearrange("p (r c) -> p r c", r=2)
                d3 = D[:, :].rearrange("p (r c) -> p r c", r=2)
                nc.vector.scalar_tensor_tensor(
                    out=d3,
                    in0=bank5b,
                    scalar=ccol(CB2M),
                    in1=r3,
                    op0=ALU.add,
                    op1=ALU.subtract,
                ).wait_op(pS, P_L2MU, "sem-ge").then_inc(vS, 1)  # V_D
                nc.vector.tensor_mul(Q[:, :], D[:, :], D[:, :]).wait_op(
                    vS, V_D, "sem-ge"
                ).then_inc(vS, 1)  # V_Q
                vector.wait_ge(aS, A_EXP)
                q3 = Q[:, :].rearrange("p (r c) -> p r c", r=2)
                p3 = P[:, 0 : 2 * RB].rearrange("p (r c) -> p r c", r=2)
                ivb = (
                    iv[:, :]
                    .rearrange("p (r c) -> p r c", r=1)
                    .broadcast_to([128, 2, RB])
                )
                nc.vector.tensor_mul(p3, q3, ivb).wait_op(
                    vS, V_Q, "sem-ge"
                ).then_inc(vS, 1)  # V_P

    return nc


def make_in_maps(inputs: dict) -> list[dict]:
    f32 = lambda a: np.ascontiguousarray(np.asarray(a, dtype=np.float32))
    bf = lambda a: np.ascontiguousarray(np.asarray(a, dtype=np.float32)).astype(
        ml_dtypes.bfloat16
    )
    x = f32(inputs["x_samples"])  # [512, 768]
    y = f32(inputs["y_samples"])  # [512, 128]
    yT = f32(y.T)  # [128, 512]

    consts = np.zeros((128, 11), np.float32)
    consts[:, CB1M : CB1M + 4] = f32(inputs["b1_mu"]).reshape(4, 128).T
    consts[:, CB2M] = f32(inputs["b2_mu"])
    consts[:, CB1L : CB1L + 4] = f32(inputs["b1_lv"]).reshape(4, 128).T
    consts[:, CB2L] = f32(inputs["b2_lv"])
    consts[:, CONE] = 1.0

    def pack_w1(w):  # [768, 512] -> [128, 3072] chunk-major bf16
        a = bf(w).reshape(KT, 128, MT, 128)  # [t, p, m, c]
        return np.ascontiguousarray(
            a.transpose(1, 2, 0, 3).reshape(128, MT * KT * 128)
        )

    def pack_w2(w):  # [512, 128] -> [128, 512]
        a = bf(w).reshape(MT, 128, Y_DIM)
        return np.ascontiguousarray(a.transpose(1, 0, 2).reshape(128, MT * Y_DIM))

    w1m_p = pack_w1(inputs["w1_mu"])
    w1l_p = pack_w1(inputs["w1_lv"])
    w2_p = np.concatenate([pack_w2(inputs["w2_mu"]), pack_w2(inputs["w2_lv"])], axis=1)

    in_maps = []
    for c in range(N_CORES):
        sl = slice(c * RB, (c + 1) * RB)
        xs = x[sl]  # [64, 768]
        xp = np.ascontiguousarray(
            xs.T.reshape(KT, 128, RB).transpose(1, 0, 2).reshape(128, KT * RB)
        ).astype(ml_dtypes.bfloat16)
        ysT = f32(y[sl].T)  # [128, 64]

        y_hi = yT.astype(ml_dtypes.bfloat16)
        y_lo = (yT - y_hi.astype(np.float32)).astype(ml_dtypes.float8_e5m2)
        early = np.zeros((128, NE), np.uint16)
        early[:, C0 : C0 + 22] = consts.view(np.uint16)
        early[:, XS0 : XS0 + KT * RB] = xp.view(np.uint16)
        early[:, YT0 : YT0 + B] = y_hi.view(np.uint16)
        early[:, YLO0 : YLO0 + B // 2] = np.ascontiguousarray(y_lo).view(np.uint16)
        ys_h = ysT.astype(ml_dtypes.bfloat16)
        ys_l = (ysT - ys_h.astype(np.float32)).astype(ml_dtypes.float8_e5m2)
        early[:, YS0 : YS0 + RB] = ys_h.view(np.uint16)
        early[:, YSLO0 : YSLO0 + RB // 2] = np.ascontiguousarray(ys_l).view(np.uint16)

        in_maps.append(
            {
                "early": early,
                "w1m": w1m_p,
                "w1l": w1l_p,
                "w2": w2_p,
            }
        )
    return in_maps


def combine(results: list[dict]) -> np.float32:
    outs = [np.asarray(results[c]["out"], np.float64)[0] for c in range(N_CORES)]
    pos = np.concatenate([o[:RB] for o in outs])
    negc = np.concatenate([o[RB : 2 * RB] + o[2 * RB : 3 * RB] for o in outs])
    C = np.log(B - 1.0 + np.exp(-20.0)) - np.log(B - 1.0)
    loss = -0.5 * (pos.mean() - negc.mean()) - C
    return np.float32(loss)


_NC_CACHE = None


def run(inputs: dict, **spmd_kwargs):
    """Build (cached), run on 8 cores, return (loss, BassKernelResults)."""
    global _NC_CACHE
    if _NC_CACHE is None:
        _NC_CACHE = build_nc()
    bkr = run_bass_kernel_spmd(
        _NC_CACHE, make_in_maps(inputs), list(range(N_CORES)), **spmd_kwargs
    )
    return combine(bkr.results), bkr


def kernel(**inputs) -> np.float32:
    loss, _ = run(inputs)
    return loss
